# revision 1
# baseline (speedup 1.0000x reference)
"""Trainium2 Bass kernel for nn_CNN_MAMBA2 (CNN + Mamba2(L=1) + MLP head).

Strategy: pure data parallel over batch (B=256 -> 32 per core x 8 cores).
Each core runs the full network on its batch shard; weights are replicated.

Layouts (per core, bh = 32 batches x 2 rows = 64 independent 1D signals):
  X    [64, 3936]   batch-major padded input (xpad[i] = x[i-25])
  Xp   [128, 7680]  position-major: Xp[p, 64*C+bh] = xpad_bh[32*C+p]
                    (built with 120 PE transposes of overlapping 128-col blocks)
  conv1: out w = 8C + j + 4*delta; lhsT packs (tap k, delta) into K=67;
         4 j-groups x 15 N=512 chunks of fp32r matmuls; maxpool(4) fused as
         DVE max over the 4 j-group PSUMs; BN+ReLU fused into evacuation.
  P1   [128, 8320]  pooled, partition = 64*delta + ci, col = (C+5)*64 + bh
                    where pooled position m = 2C + delta  (5 C-blocks zero pad)
  conv2: tap pairs (2j, 2j+1) land on the two delta halves -> K=128 packed,
         11 accumulating matmuls per N=512 chunk.
  C3in [128, 8192]  conv2 out, col = (w+4)*64 + bh (4 w-blocks zero pad)
  conv3: K=128 per tap, 9 taps x 2 co-halves, N<=512 chunks.
  H3   2 x [128, 3840]  conv3 out (v, bh); avgpool -> feature-major h [256, 32]
  Mamba2 with L=1: single scan step from h0=0 =>
         y = xin * (dt * (B.C) + D) (per head), gated RMSNorm, out_proj, MLP.
  Feature-major mamba; partition reductions/broadcasts via ones-matmuls.

Host-side prep is layout-only (transpose/reshape/pad/tile of weights); all
arithmetic (BN folding, silu, conv, matmuls, norms) happens on device.
"""

import numpy as np

import bass_rust
import concourse.bass as bass
import concourse.mybir as mybir
from concourse import masks
from concourse.tile import TileContext
from concourse.bass_utils import run_bass_kernel_spmd

F32 = mybir.dt.float32
F32R = mybir.dt.float32r
AF = mybir.ActivationFunctionType
ALU = mybir.AluOpType
AX = mybir.AxisListType

EPS = 1e-5
NCORES = 8
BSH = 32            # batches per core
BH = 64             # bh signals per core
NC1 = 120           # C blocks (conv1 output pairs / pool blocks)
XPAD = 3936


def _split_multi_waits(nc):
    """This walrus build accepts at most one sync-wait command per
    instruction; Tile's sem assignment attaches several. Hoist extra waits
    onto dedicated single-wait nops right before the instruction (same
    engine), which preserves blocking semantics."""
    n = 0
    for fn in nc.m.functions:
        for bb in fn.blocks:
            out = []
            for inst in bb.instructions:
                si = inst.sync_info
                waits = list(si.on_wait) if si is not None else []
                if len(waits) > 1:
                    for w in waits[:-1]:
                        n += 1
                        nop = mybir.InstNoOp(name=f"waitnop-{n}", ins=[], outs=[])
                        nop.engine = inst.engine
                        nop.debug = inst.debug
                        nop.sync_info = bass_rust.SyncInfo(
                            on_wait=[w], on_update=[]
                        )
                        out.append(nop)
                    si.on_wait = [waits[-1]]
                    inst.sync_info = si
                out.append(inst)
            bb.instructions = out


# --------------------------------------------------------------------------
# host-side weight layout prep (layout only: transpose / reshape / pad / tile)
# --------------------------------------------------------------------------

def _prep_weights(inp):
    f32 = np.float32
    c1w = np.asarray(inp["c1w"], f32).reshape(64, 51)
    # lhsT for conv1: K rows are input positions c relative to the 32-position
    # chunk base; column m = 128*j is absorbed by leading 4j zero rows so the
    # rhs can always start at partition 0 (PE base-partition constraint).
    w1t = np.zeros((79, 4, 128), f32)
    for j in range(4):
        for d in range(2):
            for c in range(4 * j + 16 * d, 4 * j + 16 * d + 51):
                w1t[c, j, 64 * d : 64 * d + 64] = c1w[:, c - 4 * j - 16 * d]
    w1t = w1t.reshape(79, 512)

    c2w = np.asarray(inp["c2w"], f32).reshape(128, 64, 21)
    w2t = np.zeros((128, 11, 128), f32)
    for jp in range(11):
        for d in range(2):
            t = 2 * jp + d
            if t <= 20:
                w2t[64 * d : 64 * d + 64, jp, :] = c2w[:, :, t].T

    c3w = np.asarray(inp["c3w"], f32).reshape(256, 128, 9)
    w3t = np.zeros((128, 2, 9, 128), f32)
    for hf in range(2):
        for k in range(9):
            w3t[:, hf, k, :] = c3w[128 * hf : 128 * hf + 128, :, k].T

    mw_in = np.asarray(inp["mw_in"], f32)          # [1160, 256]
    w_inT = np.zeros((128, 2, 1160), f32)
    for k in range(2):
        w_inT[:, k, :] = mw_in[:, 128 * k : 128 * k + 128].T

    mw_out = np.asarray(inp["mw_out"], f32)        # [256, 512]
    w_outT = np.zeros((128, 4, 2, 128), f32)
    for k in range(4):
        for m in range(2):
            w_outT[:, k, m, :] = mw_out[
                128 * m : 128 * m + 128, 128 * k : 128 * k + 128
            ].T

    f1w = np.asarray(inp["f1w"], f32)              # [64, 256]
    f1wT = np.zeros((128, 2, 64), f32)
    for k in range(2):
        f1wT[:, k, :] = f1w[:, 128 * k : 128 * k + 128].T

    f2wT = np.asarray(inp["f2w"], f32).reshape(1, 64).T.copy()   # [64, 1]

    def t2(a):
        return np.tile(np.asarray(a, f32), 2)

    def pd(a):
        a = np.asarray(a, f32)
        return np.pad(a, (0, 128 - a.shape[0]))

    vecs = np.zeros((128, 44), f32)
    # cols 0-4 bn gammas, 5-9 betas, 10-14 means, 15-19 vars, 20-24 pre-bias
    vecs[:, 0] = t2(inp["bn1g"]); vecs[:, 5] = t2(inp["bn1b"])
    vecs[:, 10] = t2(inp["bn1m"]); vecs[:, 15] = t2(inp["bn1v"])
    vecs[:, 20] = t2(inp["c1b"])
    vecs[:, 1] = inp["bn2g"]; vecs[:, 6] = inp["bn2b"]
    vecs[:, 11] = inp["bn2m"]; vecs[:, 16] = inp["bn2v"]
    vecs[:, 21] = inp["c2b"]
    for hf in range(2):
        s = slice(128 * hf, 128 * hf + 128)
        vecs[:, 2 + hf] = inp["bn3g"][s]; vecs[:, 7 + hf] = inp["bn3b"][s]
        vecs[:, 12 + hf] = inp["bn3m"][s]; vecs[:, 17 + hf] = inp["bn3v"][s]
        vecs[:, 22 + hf] = inp["c3b"][s]
    vecs[:, 4] = pd(inp["bn4g"]); vecs[:, 9] = pd(inp["bn4b"])
    vecs[:, 14] = pd(inp["bn4m"]); vecs[:, 19] = pd(inp["bn4v"])
    vecs[:, 24] = pd(inp["f1b"])
    vecs[0:8, 25] = inp["mdt_bias"]
    vecs[0:8, 26] = inp["mD"]
    vecs[0:1, 27] = inp["f2b"]
    mcw = np.asarray(inp["mconv_w"], f32)[:, 0, 3]
    mcb = np.asarray(inp["mconv_b"], f32)
    vecs[:, 28:33] = mcw.reshape(5, 128).T
    vecs[:, 33:38] = mcb.reshape(5, 128).T
    vecs[:, 38:42] = np.asarray(inp["mnorm_w"], f32).reshape(4, 128).T
    vecs[0:64, 42] = mcw[576:640]
    vecs[0:64, 43] = mcb[576:640]

    # constant head-expansion matrix: emat[h, 128*t + m] = 1 iff h == 2t + m//64
    emat = np.zeros((8, 512), f32)
    for t in range(4):
        emat[2 * t, 128 * t : 128 * t + 64] = 1.0
        emat[2 * t + 1, 128 * t + 64 : 128 * t + 128] = 1.0

    return {
        "w1t": w1t, "w2t": w2t.reshape(128, -1), "w3t": w3t.reshape(128, -1),
        "w_inT": w_inT.reshape(128, -1), "w_outT": w_outT.reshape(128, -1),
        "f1wT": f1wT.reshape(128, -1), "f2wT": f2wT, "vecs": vecs, "emat": emat,
    }


# --------------------------------------------------------------------------
# device kernel
# --------------------------------------------------------------------------

def _build_nc():
    nc = bass.Bass("TRN2", target_bir_lowering=False, debug=False)

    x_d = nc.dram_tensor("x", [BSH, 2, 3840], F32, kind="ExternalInput").ap()
    w1t_d = nc.dram_tensor("w1t", [79, 512], F32R, kind="ExternalInput").ap()
    w2t_d = nc.dram_tensor("w2t", [128, 11 * 128], F32R, kind="ExternalInput").ap()
    w3t_d = nc.dram_tensor("w3t", [128, 18 * 128], F32R, kind="ExternalInput").ap()
    w_inT_d = nc.dram_tensor("w_inT", [128, 2 * 1160], F32, kind="ExternalInput").ap()
    w_outT_d = nc.dram_tensor("w_outT", [128, 1024], F32, kind="ExternalInput").ap()
    f1wT_d = nc.dram_tensor("f1wT", [128, 128], F32, kind="ExternalInput").ap()
    f2wT_d = nc.dram_tensor("f2wT", [64, 1], F32, kind="ExternalInput").ap()
    vecs_d = nc.dram_tensor("vecs", [128, 44], F32, kind="ExternalInput").ap()
    emat_d = nc.dram_tensor("emat", [8, 512], F32, kind="ExternalInput").ap()
    y_d = nc.dram_tensor("y", [1, BSH], F32, kind="ExternalOutput").ap()

    with TileContext(nc) as tc:
        _body(nc, tc, x_d, w1t_d, w2t_d, w3t_d, w_inT_d, w_outT_d,
              f1wT_d, f2wT_d, vecs_d, emat_d, y_d)
    _split_multi_waits(nc)
    return nc


def _body(nc, tc, x_d, w1t_d, w2t_d, w3t_d, w_inT_d, w_outT_d,
          f1wT_d, f2wT_d, vecs_d, emat_d, y_d):
    with (
        tc.tile_pool(name="pw", bufs=1) as pw,
        tc.tile_pool(name="pmain", bufs=1) as pm,
        tc.tile_pool(name="ptmp", bufs=3) as pt,
        tc.tile_pool(name="pp", bufs=1, space="PSUM") as pp,
    ):
        # ---- X: padded batch-major input, loaded in chunks so transposes
        # can start before the whole shard lands ----
        X = pm.tile([64, XPAD], F32)
        nc.gpsimd.memset(X[:, 0:25], 0.0)
        nc.gpsimd.memset(X[:, 3865:XPAD], 0.0)
        xflat = x_d.rearrange("b h w -> (b h) w")
        xcuts = [0, 352, 640, 1600, 2720, 3840]
        for c in range(5):
            w0, w1 = xcuts[c], xcuts[c + 1]
            nc.sync.dma_start(X[:, 25 + w0 : 25 + w1], xflat[:, w0:w1])

        ident = pw.tile([64, 64], F32)
        masks.make_identity(nc, ident[:])
        w1t = pw.tile([79, 512], F32R)
        nc.sync.dma_start(w1t[:], w1t_d)
        vecs = pw.tile([128, 44], F32)
        nc.sync.dma_start(vecs[:], vecs_d)

        # ---- T / T2: position-major via PE transposes (stride 64) ----
        # T[q, 64*D + bh] = xpad_bh[64*D + q]; T2 offset by 32 positions
        T = pm.tile([128, 60 * 64], F32R)
        T2 = pm.tile([128, 60 * 64], F32R)
        P1 = pm.tile([128, 130 * 64], F32R)
        nc.gpsimd.memset(P1[:, 0:320].bitcast(F32), 0.0)
        nc.gpsimd.memset(P1[:, 8000:8320].bitcast(F32), 0.0)
        C3in = pm.tile([128, 128 * 64], F32R)
        nc.gpsimd.memset(C3in[:, 0:256].bitcast(F32), 0.0)
        nc.gpsimd.memset(C3in[:, 7936:8192].bitcast(F32), 0.0)
        H3 = [pm.tile([128, 60 * 64], F32, tag=f"h3_{i}", name=f"h3_{i}") for i in range(2)]
        havg = [pm.tile([128, BSH], F32, tag=f"havg_{i}", name=f"havg_{i}") for i in range(2)]

        def tgroup(Tt, off, g):
            nd = 8 if g < 7 else 4
            tp = pp.tile([128, 512], F32, tag="mm", bufs=2, name="tp")
            for d in range(nd):
                D = 8 * g + d
                nc.tensor.transpose(
                    tp[:, 64 * d : 64 * d + 64],
                    X[:, 64 * D + off : 64 * D + off + 128], ident[:],
                )
            nc.scalar.copy(
                Tt[:, 512 * g : 512 * g + 64 * nd], tp[:, : 64 * nd]
            )

        ones_col = pw.tile([128, 1], F32)
        nc.gpsimd.memset(ones_col[:], 1.0)
        ones_row = pw.tile([1, 128], F32)
        nc.gpsimd.memset(ones_row[:], 1.0)
        eps_col = pw.tile([1, 1], F32)
        nc.gpsimd.memset(eps_col[:], EPS)

        # remaining weights (issued after X so they don't delay transposes)
        w2t = pw.tile([128, 11 * 128], F32R)
        nc.sync.dma_start(w2t[:], w2t_d)
        w3t = pw.tile([128, 18 * 128], F32R)
        nc.sync.dma_start(w3t[:], w3t_d)
        w_inT = pw.tile([128, 2 * 1160], F32)
        nc.sync.dma_start(w_inT[:], w_inT_d)
        w_outT = pw.tile([128, 1024], F32)
        nc.sync.dma_start(w_outT[:], w_outT_d)
        f1wT = pw.tile([128, 128], F32)
        nc.sync.dma_start(f1wT[:], f1wT_d)
        f2wT = pw.tile([64, 1], F32)
        nc.sync.dma_start(f2wT[:], f2wT_d)
        emat = pw.tile([8, 512], F32)
        nc.sync.dma_start(emat[:], emat_d)
        # ---- BN scale/bias precompute: s = g/sqrt(v+eps); c = (b0-m)*s+beta
        s_all = pw.tile([128, 5], F32)
        c_all = pw.tile([128, 5], F32)
        tmpv = pw.tile([128, 5], F32)
        nc.vector.tensor_scalar_add(tmpv[:], vecs[:, 15:20], EPS)
        nc.scalar.sqrt(tmpv[:], tmpv[:])
        nc.vector.reciprocal(tmpv[:], tmpv[:])
        nc.vector.tensor_mul(s_all[:], vecs[:, 0:5], tmpv[:])
        nc.vector.tensor_sub(tmpv[:], vecs[:, 20:25], vecs[:, 10:15])
        nc.vector.tensor_mul(tmpv[:], tmpv[:], s_all[:])
        nc.vector.tensor_add(c_all[:], tmpv[:], vecs[:, 5:10])

        # ---- conv1 + maxpool(4) + bn + relu (interleaved with transposes) ----
        # out w = 8C + j + 4*delta; C = 2D (+1 odd); rhs cols (D, bh)
        p1v = P1[:].rearrange("p (c b) -> p c b", b=64)

        def conv1_chunk(n):
            cs = slice(256 * n, 256 * n + 256)
            for par in range(2):
                Tt = T if par == 0 else T2
                idx = (2 * n + par) % 3
                if idx < 2:
                    ps = pp.tile([128, 1024], F32, tag="c1", bufs=2, name="c1")
                else:
                    ps = pp.tile([128, 1024], F32, tag="acc", bufs=1, name="c1a")
                for j in range(4):
                    nc.tensor.matmul(
                        ps[:, 256 * j : 256 * j + 256],
                        w1t[:, 128 * j : 128 * j + 128],
                        Tt[0:79, cs], start=True, stop=True,
                    )
                nc.vector.tensor_reduce(
                    p1v[:, 8 * n + 5 + par : 8 * n + 13 + par : 2, :],
                    ps[:].rearrange("p (j x) -> p x j", j=4),
                    AX.X, ALU.max,
                )
            nc.scalar.activation(
                P1[:, (8 * n + 5) * 64 : (8 * n + 5) * 64 + 512],
                P1[:, (8 * n + 5) * 64 : (8 * n + 5) * 64 + 512],
                AF.Relu, bias=c_all[:, 0:1], scale=s_all[:, 0:1],
            )

        def conv2_chunk(n):
            ps = pp.tile([128, 512], F32, tag="mm", bufs=2, name="c2")
            for jp in range(11):
                nc.tensor.matmul(
                    ps[:],
                    w2t[:, 128 * jp : 128 * jp + 128],
                    P1[:, (8 * n + jp) * 64 : (8 * n + jp) * 64 + 512],
                    start=(jp == 0), stop=(jp == 10),
                )
            nc.scalar.activation(
                C3in[:, 256 + 512 * n : 256 + 512 * n + 512], ps[:],
                AF.Relu, bias=c_all[:, 1:2], scale=s_all[:, 1:2],
            )

        c3v = C3in[:].rearrange("p (w b) -> p w b", b=64)
        chunks3 = [(8 * i, 8) for i in range(7)] + [(56, 4)]

        def conv3_chunk(hf, ci):
            v0, nv = chunks3[ci]
            ps = pp.tile([128, 512], F32, tag="mm", bufs=2, name="c3")
            out_ap = ps[:, : nv * 64]
            for k in range(9):
                rhs = c3v[:, 2 * v0 + k : 2 * v0 + k + 2 * nv : 2, :]
                nc.tensor.matmul(
                    ps[:, : nv * 64],
                    w3t[:, (hf * 9 + k) * 128 : (hf * 9 + k) * 128 + 128],
                    rhs,
                    start=(k == 0), stop=(k == 8),
                )
            nc.scalar.activation(
                H3[hf][:, 64 * v0 : 64 * (v0 + nv)], out_ap,
                AF.Relu, bias=c_all[:, 2 + hf : 3 + hf],
                scale=s_all[:, 2 + hf : 3 + hf],
            )
            hv = H3[hf][:, 64 * v0 : 64 * (v0 + nv)].rearrange(
                "p (v b h) -> p b v h", v=nv, b=32, h=2
            )
            if ci == 0:
                nc.vector.tensor_reduce(havg[hf][:], hv, AX.XY, ALU.add)
            else:
                hp = pt.tile([128, BSH], F32, tag="hp", name="hp")
                nc.vector.tensor_reduce(hp[:], hv, AX.XY, ALU.add)
                nc.vector.tensor_add(havg[hf][:], havg[hf][:], hp[:])
            if ci == len(chunks3) - 1:
                nc.vector.tensor_scalar_mul(havg[hf][:], havg[hf][:], 1.0 / 120.0)

        # interleaved emission: conv1(n) -> conv2(n-3) -> conv3(hf0, ...)
        state = {"e1": 0, "e2": 0, "e3": 0}

        def pump():
            while state["e2"] <= state["e1"] - 3 and state["e2"] < 15:
                conv2_chunk(state["e2"])
                state["e2"] += 1
                while state["e3"] < 8 and 2 * state["e3"] + 3 <= state["e2"] - 1:
                    conv3_chunk(0, state["e3"])
                    state["e3"] += 1

        for g in range(8):
            tgroup(T, 0, g)
            tgroup(T2, 32, g)
            while state["e1"] <= 2 * g - 1 and state["e1"] < 15:
                conv1_chunk(state["e1"])
                state["e1"] += 1
                pump()
        while state["e1"] < 15:
            conv1_chunk(state["e1"])
            state["e1"] += 1
            pump()
        while state["e2"] < 15:
            conv2_chunk(state["e2"])
            state["e2"] += 1
            while state["e3"] < 8 and 2 * state["e3"] + 3 <= state["e2"] - 1:
                conv3_chunk(0, state["e3"])
                state["e3"] += 1
        while state["e3"] < 8:
            conv3_chunk(0, state["e3"])
            state["e3"] += 1

        for ci in range(8):
            conv3_chunk(1, ci)

        # in_proj: M-tiles (z:0-3, xBC, dt), K=2x128
        ip = pp.tile([128, 352], F32, tag="c1", bufs=2, name="ip")
        mtiles = [(10, 1152, 8), (8, 1024, 64), (9, 1088, 64)]
        mtiles += [(m, 128 * m, 128) for m in range(4, 8)]
        mtiles += [(m, 128 * m, 128) for m in range(4)]
        for m, f0, mm in mtiles:
            for k in range(2):
                nc.tensor.matmul(
                    ip[0:mm, 32 * m : 32 * m + 32],
                    w_inT[:, 1160 * k + f0 : 1160 * k + f0 + mm],
                    havg[k][:],
                    start=(k == 0), stop=(k == 1),
                )

        # ---- mamba + classifier (feature-major, batch on free dim) ----
        xcB = pt.tile([64, BSH], F32, tag="xcB")
        nc.scalar.activation(
            xcB[:], ip[0:64, 256:288], AF.Silu,
            bias=vecs[0:64, 37:38], scale=vecs[0:64, 32:33],
        )
        xcC = pt.tile([64, BSH], F32, tag="xcC")
        nc.scalar.activation(
            xcC[:], ip[0:64, 288:320], AF.Silu,
            bias=vecs[0:64, 43:44], scale=vecs[0:64, 42:43],
        )
        dts = pt.tile([8, BSH], F32, tag="dts")
        # softplus(x + b) = ln(1 + exp(x + b)) (no softplus ACT table here)
        nc.scalar.activation(
            dts[:], ip[0:8, 320:352], AF.Exp, bias=vecs[0:8, 25:26]
        )
        nc.scalar.activation(dts[:], dts[:], AF.Ln, bias=1.0)
        xc = [pt.tile([128, BSH], F32, tag=f"xc{m}", name=f"xc{m}") for m in range(4)]
        for m in range(4):
            nc.scalar.activation(
                xc[m][:], ip[:, 32 * (4 + m) : 32 * (4 + m) + 32], AF.Silu,
                bias=vecs[:, 33 + m : 34 + m], scale=vecs[:, 28 + m : 29 + m],
            )
        zsall = pt.tile([128, 4 * BSH], F32, tag="zsall")
        nc.scalar.activation(zsall[:], ip[:, 0:128], AF.Silu)
        zs = [zsall[:, 32 * m : 32 * m + 32] for m in range(4)]

        # s = sum_f Bm*Cm  (per batch scalar), via ones-matmul
        bc = pt.tile([64, BSH], F32, tag="bc")
        nc.vector.tensor_mul(bc[:], xcB[:], xcC[:])
        ps_s = pp.tile([1, BSH], F32, tag="mm", bufs=2, name="ps_s")
        nc.tensor.matmul(ps_s[:], ones_col[0:64, :], bc[:], start=True, stop=True)
        s_sb = pt.tile([1, BSH], F32, tag="s_sb")
        nc.vector.tensor_copy(s_sb[:], ps_s[:])
        ps_s8 = pp.tile([8, BSH], F32, tag="mm", bufs=2, name="ps_s8")
        nc.tensor.matmul(ps_s8[:], ones_row[0:1, 0:8], s_sb[:], start=True, stop=True)
        g = pt.tile([8, BSH], F32, tag="g")
        nc.vector.tensor_mul(g[:], dts[:], ps_s8[:])
        nc.vector.tensor_scalar_add(g[:], g[:], vecs[0:8, 26:27])

        y = [pt.tile([128, BSH], F32, tag=f"y{t}", name=f"y{t}") for t in range(4)]
        ps_ms = pp.tile([1, BSH], F32, tag="c1", bufs=2, name="ps_ms")
        for t in range(4):
            ge = pp.tile([128, BSH], F32, tag="mm", bufs=2, name="ge")
            nc.tensor.matmul(ge[:], emat[:, 128 * t : 128 * t + 128], g[:],
                             start=True, stop=True)
            nc.vector.tensor_mul(y[t][:], xc[t][:], ge[:])
            nc.vector.tensor_mul(y[t][:], y[t][:], zs[t])
            sq = pt.tile([128, BSH], F32, tag="sq")
            nc.vector.tensor_mul(sq[:], y[t][:], y[t][:])
            nc.tensor.matmul(ps_ms[:], ones_col[:], sq[:],
                             start=(t == 0), stop=(t == 3))
        sd = pt.tile([1, BSH], F32, tag="sd")
        nc.scalar.activation(sd[:], ps_ms[:], AF.Sqrt,
                             bias=eps_col[:], scale=1.0 / 512.0)
        rinv = pt.tile([1, BSH], F32, tag="rinv")
        nc.vector.reciprocal(rinv[:], sd[:])
        ps_rb = pp.tile([128, BSH], F32, tag="mm", bufs=2, name="ps_rb")
        nc.tensor.matmul(ps_rb[:], ones_row[:], rinv[:], start=True, stop=True)

        yn = [pt.tile([128, BSH], F32, tag=f"yn{t}", name=f"yn{t}") for t in range(4)]
        for t in range(4):
            nc.vector.tensor_mul(yn[t][:], y[t][:], ps_rb[:])
            nc.vector.tensor_scalar_mul(yn[t][:], yn[t][:],
                                        vecs[:, 38 + t : 39 + t])

        # out_proj [256,512] @ yn -> o [256, 32] (2 M-tiles in one psum)
        ps_o = pp.tile([128, 64], F32, tag="mm", bufs=2, name="ps_o")
        for m in range(2):
            for k in range(4):
                nc.tensor.matmul(
                    ps_o[:, 32 * m : 32 * m + 32],
                    w_outT[:, (k * 2 + m) * 128 : (k * 2 + m) * 128 + 128],
                    yn[k][:],
                    start=(k == 0), stop=(k == 3),
                )
        o_sb = pt.tile([128, 64], F32, tag="o_sb")
        nc.vector.tensor_copy(o_sb[:], ps_o[:])

        # fc1 + bn4 + relu
        ps_f1 = pp.tile([64, BSH], F32, tag="c1", bufs=2, name="ps_f1")
        for k in range(2):
            nc.tensor.matmul(
                ps_f1[:], f1wT[:, 64 * k : 64 * k + 64],
                o_sb[:, 32 * k : 32 * k + 32],
                start=(k == 0), stop=(k == 1),
            )
        o1 = pt.tile([64, BSH], F32, tag="o1")
        nc.scalar.activation(o1[:], ps_f1[:], AF.Relu,
                             bias=c_all[0:64, 4:5], scale=s_all[0:64, 4:5])

        # fc2
        ps_f2 = pp.tile([1, BSH], F32, tag="c1", bufs=2, name="ps_f2")
        nc.tensor.matmul(ps_f2[:], f2wT[:], o1[:], start=True, stop=True)
        ores = pt.tile([1, BSH], F32, tag="ores")
        nc.scalar.activation(ores[:], ps_f2[:], AF.Identity,
                             bias=vecs[0:1, 27:28])
        nc.sync.dma_start(y_d, ores[:])


_NC_CACHE = []


def kernel(**inputs):
    if not _NC_CACHE:
        _NC_CACHE.append(_build_nc())
    nc = _NC_CACHE[0]
    w = _prep_weights(inputs)
    x = np.asarray(inputs["x"], np.float32)
    in_maps = []
    for c in range(NCORES):
        m = dict(w)
        m["x"] = np.ascontiguousarray(x[c * BSH : (c + 1) * BSH])
        in_maps.append(m)
    res = run_bass_kernel_spmd(nc, in_maps, list(range(NCORES))).results
    out = np.concatenate([res[c]["y"].reshape(BSH, 1) for c in range(NCORES)], 0)
    return out



# revision 8
# speedup vs baseline: 1.2177x; 1.2177x over previous
"""Trainium2 Bass kernel for nn_CNN_MAMBA2 (CNN + Mamba2(L=1) + MLP head).

Strategy: pure data parallel over batch (B=256 -> 32 per core x 8 cores).
Each core runs the full network on its batch shard; weights are replicated.

Layouts (per core, bh = 32 batches x 2 rows = 64 independent 1D signals):
  X    [64, 3936]   batch-major padded input (xpad[i] = x[i-25])
  Xp   [128, 7680]  position-major: built with 120 PE transposes.
  conv1: fp32r matmuls as before; maxpool(4) as DVE 4:1 tensor_reduce from
         PSUM into bf16 staging M1; BN+ReLU+fp8 cast on the Pool engine
         (2 tensor_scalar ops) writing P1.
  P1   [128, 8320]  pooled, fp8e4 (x16 scale), partition = 64*delta + ci
  conv2: fp8e4 DoubleRow matmuls. Tap pairs (jp, jp+1) are 64-col shifts of
         P1, expressed as one overlapping AP with a 2-wide k-tile dim.
         Weights x32 in fp8, optionally split hi+lo for precision.
  C3in [128, 8192]  conv2 out, fp8e4 (x16 scale), written by Act evacuation
  conv3: fp8e4 DoubleRow, same trick (taps are 64-col shifts).
  H3   2 x [128, 3840] bf16 conv3 out; avgpool = DVE reduce (2x mode on
         bf16) -> feature-major h [256, 32] bf16
  Mamba2 with L=1: single scan step from h0=0 =>
         y = xin * (dt * (B.C) + D) (per head), gated RMSNorm, out_proj, MLP.
  Feature-major mamba; partition reductions/broadcasts via ones-matmuls.

Host-side prep is layout-only plus dtype casts and exact power-of-2
scalings of weights (fp8 range placement); BN folding and the matching
1/2^k compensation happen on device via constant multiplier columns.
"""

import numpy as np

import bass_rust
import concourse.bass as bass
import concourse.mybir as mybir
from concourse import masks
from concourse.ap import AP
from concourse.tile import TileContext
from concourse.bass_utils import run_bass_kernel_spmd

F32 = mybir.dt.float32
F32R = mybir.dt.float32r
BF16 = mybir.dt.bfloat16
F8 = mybir.dt.float8e4
AF = mybir.ActivationFunctionType
ALU = mybir.AluOpType
AX = mybir.AxisListType
PM = mybir.MatmulPerfMode

EPS = 1e-5
NCORES = 8
BSH = 32            # batches per core
BH = 64             # bh signals per core
NC1 = 120           # C blocks (conv1 output pairs / pool blocks)
XPAD = 3936

DEBUG_DUMPS = False

# hi+lo fp8 weight split per conv (False = single fp8 chain, 2x faster on
# PE but ~3.6% weight-quantization error; True = ~0.1%)
W2_SPLIT = True
W3_SPLIT = True
NCH2 = 2 if W2_SPLIT else 1
NCH3 = 2 if W3_SPLIT else 1
S1 = 16.0           # P1 fp8 scale
S3A = 16.0          # C3in fp8 scale
S2W = 32.0          # conv2 weight fp8 scale
S3W = 32.0          # conv3 weight fp8 scale

NP8 = mybir.dt.np(F8)


def _split_multi_waits(nc):
    """This walrus build accepts at most one sync-wait command per
    instruction; Tile's sem assignment attaches several. Hoist extra waits
    onto dedicated single-wait nops right before the instruction (same
    engine), which preserves blocking semantics."""
    n = 0
    for fn in nc.m.functions:
        for bb in fn.blocks:
            out = []
            for inst in bb.instructions:
                si = inst.sync_info
                waits = list(si.on_wait) if si is not None else []
                if len(waits) > 1:
                    for w in waits[:-1]:
                        n += 1
                        nop = mybir.InstNoOp(name=f"waitnop-{n}", ins=[], outs=[])
                        nop.engine = inst.engine
                        nop.debug = inst.debug
                        nop.sync_info = bass_rust.SyncInfo(
                            on_wait=[w], on_update=[]
                        )
                        out.append(nop)
                    si.on_wait = [waits[-1]]
                    inst.sync_info = si
                out.append(inst)
            bb.instructions = out


def _sview(t, col0, dims):
    """Overlapping strided free-dim view of a [128, W] tile: dims is a list
    of [stride, size] free dims, partition dim kept."""
    base = t[:, col0 : col0 + 1]
    return AP(base.tensor, base.offset, [list(base.ap[0])] + dims)


# --------------------------------------------------------------------------
# host-side weight prep (layout / cast / exact power-of-2 scaling only)
# --------------------------------------------------------------------------

def _quant_pairs(full, scale, n_pairs, split):
    """full: [128, ntap, 128] f32. Returns [128, nch * n_pairs * 2 * 128]
    fp8 with tap pairs (2q, 2q+1) stacked as DoubleRow k-tiles, missing
    taps zero, lo chain = residual after hi quantization."""
    k, ntap, m = full.shape
    sc = (full * scale).astype(np.float32)
    hi = sc.astype(NP8)
    chains = [hi]
    if split:
        lo = (sc - hi.astype(np.float32)).astype(NP8)
        chains.append(lo)
    out = np.zeros((k, len(chains), n_pairs, 2, m), NP8)
    for c, arr in enumerate(chains):
        for q in range(n_pairs):
            for i in range(2):
                t = 2 * q + i
                if t < ntap:
                    out[:, c, q, i, :] = arr[:, t, :]
    return out.reshape(k, -1)


def _prep_weights(inp):
    f32 = np.float32
    c1w = np.asarray(inp["c1w"], f32).reshape(64, 51)
    # lhsT for conv1: K rows are input positions c relative to the 32-position
    # chunk base; column m = 128*j is absorbed by leading 4j zero rows so the
    # rhs can always start at partition 0 (PE base-partition constraint).
    w1t = np.zeros((79, 4, 128), f32)
    for j in range(4):
        for d in range(2):
            for c in range(4 * j + 16 * d, 4 * j + 16 * d + 51):
                w1t[c, j, 64 * d : 64 * d + 64] = c1w[:, c - 4 * j - 16 * d]
    w1t = w1t.reshape(79, 512)

    c2w = np.asarray(inp["c2w"], f32).reshape(128, 64, 21)
    w2full = np.zeros((128, 11, 128), f32)
    for jp in range(11):
        for d in range(2):
            t = 2 * jp + d
            if t <= 20:
                w2full[64 * d : 64 * d + 64, jp, :] = c2w[:, :, t].T
    w2t = _quant_pairs(w2full, S2W, 6, W2_SPLIT)

    c3w = np.asarray(inp["c3w"], f32).reshape(256, 128, 9)
    w3full = np.zeros((128, 2 * 9, 128), f32)
    for hf in range(2):
        for k in range(9):
            w3full[:, 9 * hf + k, :] = c3w[128 * hf : 128 * hf + 128, :, k].T
    # pack per hf separately (pairs within each hf's 9 taps)
    w3t = np.concatenate(
        [
            _quant_pairs(w3full[:, 9 * hf : 9 * hf + 9, :], S3W, 5, W3_SPLIT)
            for hf in range(2)
        ],
        axis=1,
    )

    mw_in = np.asarray(inp["mw_in"], f32)          # [1160, 256]
    w_inT = np.zeros((128, 2, 1160), f32)
    for k in range(2):
        w_inT[:, k, :] = mw_in[:, 128 * k : 128 * k + 128].T
    w_inT = w_inT.astype(np.dtype(mybir.dt.np(BF16)))

    mw_out = np.asarray(inp["mw_out"], f32)        # [256, 512]
    w_outT = np.zeros((128, 4, 2, 128), f32)
    for k in range(4):
        for m in range(2):
            w_outT[:, k, m, :] = mw_out[
                128 * m : 128 * m + 128, 128 * k : 128 * k + 128
            ].T

    f1w = np.asarray(inp["f1w"], f32)              # [64, 256]
    f1wT = np.zeros((128, 2, 64), f32)
    for k in range(2):
        f1wT[:, k, :] = f1w[:, 128 * k : 128 * k + 128].T

    f2wT = np.asarray(inp["f2w"], f32).reshape(1, 64).T.copy()   # [64, 1]

    def t2(a):
        return np.tile(np.asarray(a, f32), 2)

    def pd(a):
        a = np.asarray(a, f32)
        return np.pad(a, (0, 128 - a.shape[0]))

    vecs = np.zeros((128, 54), f32)
    # cols 0-4 bn gammas, 5-9 betas, 10-14 means, 15-19 vars, 20-24 pre-bias
    vecs[:, 0] = t2(inp["bn1g"]); vecs[:, 5] = t2(inp["bn1b"])
    vecs[:, 10] = t2(inp["bn1m"]); vecs[:, 15] = t2(inp["bn1v"])
    vecs[:, 20] = t2(inp["c1b"])
    vecs[:, 1] = inp["bn2g"]; vecs[:, 6] = inp["bn2b"]
    vecs[:, 11] = inp["bn2m"]; vecs[:, 16] = inp["bn2v"]
    vecs[:, 21] = inp["c2b"]
    for hf in range(2):
        s = slice(128 * hf, 128 * hf + 128)
        vecs[:, 2 + hf] = inp["bn3g"][s]; vecs[:, 7 + hf] = inp["bn3b"][s]
        vecs[:, 12 + hf] = inp["bn3m"][s]; vecs[:, 17 + hf] = inp["bn3v"][s]
        vecs[:, 22 + hf] = inp["c3b"][s]
    vecs[:, 4] = pd(inp["bn4g"]); vecs[:, 9] = pd(inp["bn4b"])
    vecs[:, 14] = pd(inp["bn4m"]); vecs[:, 19] = pd(inp["bn4v"])
    vecs[:, 24] = pd(inp["f1b"])
    vecs[0:8, 25] = inp["mdt_bias"]
    vecs[0:8, 26] = inp["mD"]
    vecs[0:1, 27] = inp["f2b"]
    mcw = np.asarray(inp["mconv_w"], f32)[:, 0, 3]
    mcb = np.asarray(inp["mconv_b"], f32)
    vecs[:, 28:33] = mcw.reshape(5, 128).T
    vecs[:, 33:38] = mcb.reshape(5, 128).T
    vecs[:, 38:42] = np.asarray(inp["mnorm_w"], f32).reshape(4, 128).T
    vecs[0:64, 42] = mcw[576:640]
    vecs[0:64, 43] = mcb[576:640]
    # constant fp8-scale compensation multipliers for BN scale/shift
    vecs[:, 44:49] = np.array(
        [S1, S3A / (S1 * S2W), 1.0 / (S3A * S3W), 1.0 / (S3A * S3W), 1.0],
        f32,
    )[None, :]
    vecs[:, 49:54] = np.array([S1, S3A, 1.0, 1.0, 1.0], f32)[None, :]

    # constant head-expansion matrix: emat[h, 128*t + m] = 1 iff h == 2t + m//64
    emat = np.zeros((8, 512), f32)
    for t in range(4):
        emat[2 * t, 128 * t : 128 * t + 64] = 1.0
        emat[2 * t + 1, 128 * t + 64 : 128 * t + 128] = 1.0

    return {
        "w1t": w1t, "w2t": w2t, "w3t": w3t,
        "w_inT": w_inT.reshape(128, -1), "w_outT": w_outT.reshape(128, -1),
        "f1wT": f1wT.reshape(128, -1), "f2wT": f2wT, "vecs": vecs, "emat": emat,
    }


# --------------------------------------------------------------------------
# device kernel
# --------------------------------------------------------------------------

def _build_nc():
    nc = bass.Bass("TRN2", target_bir_lowering=False, debug=False)

    x_d = nc.dram_tensor("x", [BSH, 2, 3840], F32, kind="ExternalInput").ap()
    w1t_d = nc.dram_tensor("w1t", [79, 512], F32R, kind="ExternalInput").ap()
    w2t_d = nc.dram_tensor("w2t", [128, NCH2 * 6 * 256], F8,
                           kind="ExternalInput").ap()
    w3t_d = nc.dram_tensor("w3t", [128, NCH3 * 2 * 5 * 256], F8,
                           kind="ExternalInput").ap()
    w_inT_d = nc.dram_tensor("w_inT", [128, 2 * 1160], BF16,
                             kind="ExternalInput").ap()
    w_outT_d = nc.dram_tensor("w_outT", [128, 1024], F32, kind="ExternalInput").ap()
    f1wT_d = nc.dram_tensor("f1wT", [128, 128], F32, kind="ExternalInput").ap()
    f2wT_d = nc.dram_tensor("f2wT", [64, 1], F32, kind="ExternalInput").ap()
    vecs_d = nc.dram_tensor("vecs", [128, 54], F32, kind="ExternalInput").ap()
    emat_d = nc.dram_tensor("emat", [8, 512], F32, kind="ExternalInput").ap()
    y_d = nc.dram_tensor("y", [1, BSH], F32, kind="ExternalOutput").ap()
    dbg = {}
    if DEBUG_DUMPS:
        dbg["P1"] = nc.dram_tensor("dbg_P1", [128, 130 * 64], F8,
                                   kind="ExternalOutput").ap()
        dbg["C3in"] = nc.dram_tensor("dbg_C3in", [128, 128 * 64], F8,
                                     kind="ExternalOutput").ap()
        dbg["H30"] = nc.dram_tensor("dbg_H30", [128, 60 * 64], BF16,
                                    kind="ExternalOutput").ap()
        dbg["havg0"] = nc.dram_tensor("dbg_havg0", [128, BSH], BF16,
                                      kind="ExternalOutput").ap()
        dbg["havg1"] = nc.dram_tensor("dbg_havg1", [128, BSH], BF16,
                                      kind="ExternalOutput").ap()

    with TileContext(nc) as tc:
        _body(nc, tc, x_d, w1t_d, w2t_d, w3t_d, w_inT_d, w_outT_d,
              f1wT_d, f2wT_d, vecs_d, emat_d, y_d, dbg)
    _split_multi_waits(nc)
    return nc


def _body(nc, tc, x_d, w1t_d, w2t_d, w3t_d, w_inT_d, w_outT_d,
          f1wT_d, f2wT_d, vecs_d, emat_d, y_d, dbg=None):
    with (
        tc.tile_pool(name="pw", bufs=1) as pw,
        tc.tile_pool(name="pmain", bufs=1) as pm,
        tc.tile_pool(name="ptmp", bufs=3) as pt,
        tc.tile_pool(name="pp", bufs=1, space="PSUM") as pp,
    ):
        # ---- X: padded batch-major input, loaded in chunks so transposes
        # can start before the whole shard lands ----
        X = pm.tile([64, XPAD], F32)
        nc.gpsimd.memset(X[:, 0:25], 0.0)
        nc.gpsimd.memset(X[:, 3865:XPAD], 0.0)
        xflat = x_d.rearrange("b h w -> (b h) w")
        xcuts = [0, 352, 640, 1600, 2720, 3840]
        for c in range(5):
            w0, w1 = xcuts[c], xcuts[c + 1]
            nc.sync.dma_start(X[:, 25 + w0 : 25 + w1], xflat[:, w0:w1])

        ident = pw.tile([64, 64], F32)
        masks.make_identity(nc, ident[:])
        w1t = pw.tile([79, 512], F32R)
        nc.sync.dma_start(w1t[:], w1t_d)
        vecs = pw.tile([128, 54], F32)
        nc.sync.dma_start(vecs[:], vecs_d)

        # ---- T / T2: position-major via PE transposes (stride 64) ----
        T = pm.tile([128, 60 * 64], F32R)
        T2 = pm.tile([128, 60 * 64], F32R)
        P1 = pm.tile([128, 130 * 64], F8)
        nc.gpsimd.memset(P1[:, 0:320], 0.0)
        nc.gpsimd.memset(P1[:, 8000:8320], 0.0)
        C3in = pm.tile([128, 128 * 64], F8)
        nc.gpsimd.memset(C3in[:, 0:256], 0.0)
        nc.gpsimd.memset(C3in[:, 7936:8192], 0.0)
        H3 = [pm.tile([128, 60 * 64], BF16, tag=f"h3_{i}", name=f"h3_{i}")
              for i in range(2)]
        havg = [pm.tile([128, BSH], BF16, tag=f"havg_{i}", name=f"havg_{i}")
                for i in range(2)]

        def tgroup(Tt, off, g):
            nd = 8 if g < 7 else 4
            tp = pp.tile([128, 512], F32, tag="mm", bufs=2, name="tp")
            for d in range(nd):
                D = 8 * g + d
                nc.tensor.transpose(
                    tp[:, 64 * d : 64 * d + 64],
                    X[:, 64 * D + off : 64 * D + off + 128], ident[:],
                )
            nc.scalar.copy(
                Tt[:, 512 * g : 512 * g + 64 * nd], tp[:, : 64 * nd]
            )

        ones_col = pw.tile([128, 1], F32)
        nc.gpsimd.memset(ones_col[:], 1.0)
        ones_row = pw.tile([1, 128], F32)
        nc.gpsimd.memset(ones_row[:], 1.0)
        eps_col = pw.tile([1, 1], F32)
        nc.gpsimd.memset(eps_col[:], EPS)

        # remaining weights (issued after X so they don't delay transposes)
        w2t = pw.tile([128, NCH2 * 6 * 256], F8)
        nc.sync.dma_start(w2t[:], w2t_d)
        w3t = pw.tile([128, NCH3 * 2 * 5 * 256], F8)
        nc.sync.dma_start(w3t[:], w3t_d)
        w_inT = pw.tile([128, 2 * 1160], BF16)
        nc.sync.dma_start(w_inT[:], w_inT_d)
        w_outT = pw.tile([128, 1024], F32)
        nc.sync.dma_start(w_outT[:], w_outT_d)
        f1wT = pw.tile([128, 128], F32)
        nc.sync.dma_start(f1wT[:], f1wT_d)
        f2wT = pw.tile([64, 1], F32)
        nc.sync.dma_start(f2wT[:], f2wT_d)
        emat = pw.tile([8, 512], F32)
        nc.sync.dma_start(emat[:], emat_d)
        # ---- BN scale/bias precompute: s = g/sqrt(v+eps); c = (b0-m)*s+beta
        # then fp8-range compensation via constant multiplier columns
        s_all = pw.tile([128, 5], F32)
        c_all = pw.tile([128, 5], F32)
        tmpv = pw.tile([128, 5], F32)
        nc.vector.tensor_scalar_add(tmpv[:], vecs[:, 15:20], EPS)
        nc.scalar.sqrt(tmpv[:], tmpv[:])
        nc.vector.reciprocal(tmpv[:], tmpv[:])
        nc.vector.tensor_mul(s_all[:], vecs[:, 0:5], tmpv[:])
        nc.vector.tensor_sub(tmpv[:], vecs[:, 20:25], vecs[:, 10:15])
        nc.vector.tensor_mul(tmpv[:], tmpv[:], s_all[:])
        nc.vector.tensor_add(c_all[:], tmpv[:], vecs[:, 5:10])
        nc.vector.tensor_mul(s_all[:], s_all[:], vecs[:, 44:49])
        nc.vector.tensor_mul(c_all[:], c_all[:], vecs[:, 49:54])

        # ---- conv1 + maxpool(4) + bn + relu ----
        # out w = 8C + j + 4*delta; C = 2D (+1 odd); rhs cols (D, bh)
        # maxpool: DVE 4:1 reduce PSUM -> M1 bf16; BN+ReLU+fp8: Pool 2 ops
        def conv1_chunk(n):
            M1 = pt.tile([128, 512], BF16, tag="m1", name="m1")
            m1v = M1[:].rearrange("p (c b) -> p c b", b=64)
            for par in range(2):
                Tt = T if par == 0 else T2
                idx = (2 * n + par) % 3
                if idx < 2:
                    ps = pp.tile([128, 1024], F32, tag="c1", bufs=2, name="c1")
                else:
                    ps = pp.tile([128, 1024], F32, tag="acc", bufs=1, name="c1a")
                cs = slice(256 * n, 256 * n + 256)
                for j in range(4):
                    nc.tensor.matmul(
                        ps[:, 256 * j : 256 * j + 256],
                        w1t[:, 128 * j : 128 * j + 128],
                        Tt[0:79, cs], start=True, stop=True,
                    )
                nc.vector.tensor_reduce(
                    m1v[:, par:8:2, :],
                    ps[:].rearrange("p (j x) -> p x j", j=4),
                    AX.X, ALU.max,
                )
            nc.gpsimd.tensor_scalar(
                M1[:], M1[:], s_all[:, 0:1], c_all[:, 0:1], ALU.mult, ALU.add
            )
            nc.gpsimd.tensor_scalar(
                P1[:, (8 * n + 5) * 64 : (8 * n + 5) * 64 + 512],
                M1[:], 0.0, None, ALU.max,
            )

        def conv2_chunk(n):
            ps = pp.tile([128, 512], F32, tag="mm", bufs=2, name="c2")
            nmm = NCH2 * 6
            mi = 0
            for ch in range(NCH2):
                for q in range(6):
                    stride = 64 if q < 5 else 0
                    rhs = _sview(P1, (8 * n + 2 * q) * 64,
                                 [[stride, 2], [1, 512]])
                    lhsT = w2t[:, (ch * 6 + q) * 256 : (ch * 6 + q) * 256 + 256]
                    nc.tensor.matmul(
                        ps[:], lhsT.rearrange("p (i m) -> p i m", i=2), rhs,
                        start=(mi == 0), stop=(mi == nmm - 1),
                        perf_mode=PM.DoubleRow,
                    )
                    mi += 1
            nc.scalar.activation(
                C3in[:, 256 + 512 * n : 256 + 512 * n + 512], ps[:],
                AF.Relu, bias=c_all[:, 1:2], scale=s_all[:, 1:2],
            )

        chunks3 = [(8 * i, 8) for i in range(7)] + [(56, 4)]

        def conv3_chunk(hf, ci):
            v0, nv = chunks3[ci]
            ps = pp.tile([128, 512], F32, tag="mm", bufs=2, name="c3")
            out_ap = ps[:, : nv * 64]
            nmm = NCH3 * 5
            mi = 0
            for ch in range(NCH3):
                for q in range(5):
                    rhs = _sview(C3in, (2 * v0 + 2 * q) * 64,
                                 [[64, 2], [128, nv], [1, 64]])
                    off = ((hf * NCH3 + ch) * 5 + q) * 256
                    lhsT = w3t[:, off : off + 256]
                    nc.tensor.matmul(
                        out_ap, lhsT.rearrange("p (i m) -> p i m", i=2), rhs,
                        start=(mi == 0), stop=(mi == nmm - 1),
                        perf_mode=PM.DoubleRow,
                    )
                    mi += 1
            nc.scalar.activation(
                H3[hf][:, 64 * v0 : 64 * (v0 + nv)], out_ap,
                AF.Relu, bias=c_all[:, 2 + hf : 3 + hf],
                scale=s_all[:, 2 + hf : 3 + hf],
            )
            hv = H3[hf][:, 64 * v0 : 64 * (v0 + nv)].rearrange(
                "p (v b h) -> p b v h", v=nv, b=32, h=2
            )
            with nc.allow_low_precision(reason="avgpool partials in bf16"):
                if ci == 0:
                    nc.vector.tensor_reduce(havg[hf][:], hv, AX.XY, ALU.add)
                else:
                    hp = pt.tile([128, BSH], BF16, tag="hp", name="hp")
                    nc.vector.tensor_reduce(hp[:], hv, AX.XY, ALU.add)
                    nc.vector.tensor_add(havg[hf][:], havg[hf][:], hp[:])
                if ci == len(chunks3) - 1:
                    nc.vector.tensor_scalar_mul(
                        havg[hf][:], havg[hf][:], 1.0 / 120.0
                    )

        # interleaved emission: conv1(n) -> conv2(n-3) -> conv3(hf0, ...)
        state = {"e1": 0, "e2": 0, "e3": 0}

        def pump():
            while state["e2"] <= state["e1"] - 3 and state["e2"] < 15:
                conv2_chunk(state["e2"])
                state["e2"] += 1
                while state["e3"] < 8 and 2 * state["e3"] + 3 <= state["e2"] - 1:
                    conv3_chunk(0, state["e3"])
                    state["e3"] += 1

        for g in range(8):
            tgroup(T, 0, g)
            tgroup(T2, 32, g)
            while state["e1"] <= 2 * g - 1 and state["e1"] < 15:
                conv1_chunk(state["e1"])
                state["e1"] += 1
                pump()
        while state["e1"] < 15:
            conv1_chunk(state["e1"])
            state["e1"] += 1
            pump()
        while state["e2"] < 15:
            conv2_chunk(state["e2"])
            state["e2"] += 1
            while state["e3"] < 8 and 2 * state["e3"] + 3 <= state["e2"] - 1:
                conv3_chunk(0, state["e3"])
                state["e3"] += 1
        while state["e3"] < 8:
            conv3_chunk(0, state["e3"])
            state["e3"] += 1

        for ci in range(8):
            conv3_chunk(1, ci)

        # in_proj: M-tiles (z:0-3, xBC, dt), K=2x128, bf16 weights/acts
        ip = pp.tile([128, 352], F32, tag="c1", bufs=2, name="ip")
        mtiles = [(10, 1152, 8), (8, 1024, 64), (9, 1088, 64)]
        mtiles += [(m, 128 * m, 128) for m in range(4, 8)]
        mtiles += [(m, 128 * m, 128) for m in range(4)]
        for m, f0, mm in mtiles:
            for k in range(2):
                nc.tensor.matmul(
                    ip[0:mm, 32 * m : 32 * m + 32],
                    w_inT[:, 1160 * k + f0 : 1160 * k + f0 + mm],
                    havg[k][:],
                    start=(k == 0), stop=(k == 1),
                )

        # ---- mamba + classifier (feature-major, batch on free dim) ----
        xcB = pt.tile([64, BSH], F32, tag="xcB")
        nc.scalar.activation(
            xcB[:], ip[0:64, 256:288], AF.Silu,
            bias=vecs[0:64, 37:38], scale=vecs[0:64, 32:33],
        )
        xcC = pt.tile([64, BSH], F32, tag="xcC")
        nc.scalar.activation(
            xcC[:], ip[0:64, 288:320], AF.Silu,
            bias=vecs[0:64, 43:44], scale=vecs[0:64, 42:43],
        )
        dts = pt.tile([8, BSH], F32, tag="dts")
        # softplus(x + b) = ln(1 + exp(x + b)) (no softplus ACT table here)
        nc.scalar.activation(
            dts[:], ip[0:8, 320:352], AF.Exp, bias=vecs[0:8, 25:26]
        )
        nc.scalar.activation(dts[:], dts[:], AF.Ln, bias=1.0)
        xc = [pt.tile([128, BSH], F32, tag=f"xc{m}", name=f"xc{m}") for m in range(4)]
        for m in range(4):
            nc.scalar.activation(
                xc[m][:], ip[:, 32 * (4 + m) : 32 * (4 + m) + 32], AF.Silu,
                bias=vecs[:, 33 + m : 34 + m], scale=vecs[:, 28 + m : 29 + m],
            )
        zsall = pt.tile([128, 4 * BSH], F32, tag="zsall")
        nc.scalar.activation(zsall[:], ip[:, 0:128], AF.Silu)
        zs = [zsall[:, 32 * m : 32 * m + 32] for m in range(4)]

        # s = sum_f Bm*Cm  (per batch scalar), via ones-matmul
        bc = pt.tile([64, BSH], F32, tag="bc")
        nc.vector.tensor_mul(bc[:], xcB[:], xcC[:])
        ps_s = pp.tile([1, BSH], F32, tag="mm", bufs=2, name="ps_s")
        nc.tensor.matmul(ps_s[:], ones_col[0:64, :], bc[:], start=True, stop=True)
        s_sb = pt.tile([1, BSH], F32, tag="s_sb")
        nc.vector.tensor_copy(s_sb[:], ps_s[:])
        ps_s8 = pp.tile([8, BSH], F32, tag="mm", bufs=2, name="ps_s8")
        nc.tensor.matmul(ps_s8[:], ones_row[0:1, 0:8], s_sb[:], start=True, stop=True)
        g = pt.tile([8, BSH], F32, tag="g")
        nc.vector.tensor_mul(g[:], dts[:], ps_s8[:])
        nc.vector.tensor_scalar_add(g[:], g[:], vecs[0:8, 26:27])

        y = [pt.tile([128, BSH], F32, tag=f"y{t}", name=f"y{t}") for t in range(4)]
        ps_ms = pp.tile([1, BSH], F32, tag="c1", bufs=2, name="ps_ms")
        for t in range(4):
            ge = pp.tile([128, BSH], F32, tag="mm", bufs=2, name="ge")
            nc.tensor.matmul(ge[:], emat[:, 128 * t : 128 * t + 128], g[:],
                             start=True, stop=True)
            nc.vector.tensor_mul(y[t][:], xc[t][:], ge[:])
            nc.vector.tensor_mul(y[t][:], y[t][:], zs[t])
            sq = pt.tile([128, BSH], F32, tag="sq")
            nc.vector.tensor_mul(sq[:], y[t][:], y[t][:])
            nc.tensor.matmul(ps_ms[:], ones_col[:], sq[:],
                             start=(t == 0), stop=(t == 3))
        sd = pt.tile([1, BSH], F32, tag="sd")
        nc.scalar.activation(sd[:], ps_ms[:], AF.Sqrt,
                             bias=eps_col[:], scale=1.0 / 512.0)
        rinv = pt.tile([1, BSH], F32, tag="rinv")
        nc.vector.reciprocal(rinv[:], sd[:])
        ps_rb = pp.tile([128, BSH], F32, tag="mm", bufs=2, name="ps_rb")
        nc.tensor.matmul(ps_rb[:], ones_row[:], rinv[:], start=True, stop=True)

        yn = [pt.tile([128, BSH], F32, tag=f"yn{t}", name=f"yn{t}") for t in range(4)]
        for t in range(4):
            nc.vector.tensor_mul(yn[t][:], y[t][:], ps_rb[:])
            nc.vector.tensor_scalar_mul(yn[t][:], yn[t][:],
                                        vecs[:, 38 + t : 39 + t])

        # out_proj [256,512] @ yn -> o [256, 32] (2 M-tiles in one psum)
        ps_o = pp.tile([128, 64], F32, tag="mm", bufs=2, name="ps_o")
        for m in range(2):
            for k in range(4):
                nc.tensor.matmul(
                    ps_o[:, 32 * m : 32 * m + 32],
                    w_outT[:, (k * 2 + m) * 128 : (k * 2 + m) * 128 + 128],
                    yn[k][:],
                    start=(k == 0), stop=(k == 3),
                )
        o_sb = pt.tile([128, 64], F32, tag="o_sb")
        nc.vector.tensor_copy(o_sb[:], ps_o[:])

        # fc1 + bn4 + relu
        ps_f1 = pp.tile([64, BSH], F32, tag="c1", bufs=2, name="ps_f1")
        for k in range(2):
            nc.tensor.matmul(
                ps_f1[:], f1wT[:, 64 * k : 64 * k + 64],
                o_sb[:, 32 * k : 32 * k + 32],
                start=(k == 0), stop=(k == 1),
            )
        o1 = pt.tile([64, BSH], F32, tag="o1")
        nc.scalar.activation(o1[:], ps_f1[:], AF.Relu,
                             bias=c_all[0:64, 4:5], scale=s_all[0:64, 4:5])

        # fc2
        ps_f2 = pp.tile([1, BSH], F32, tag="c1", bufs=2, name="ps_f2")
        nc.tensor.matmul(ps_f2[:], f2wT[:], o1[:], start=True, stop=True)
        ores = pt.tile([1, BSH], F32, tag="ores")
        nc.scalar.activation(ores[:], ps_f2[:], AF.Identity,
                             bias=vecs[0:1, 27:28])
        nc.sync.dma_start(y_d, ores[:])
        if dbg:
            nc.sync.dma_start(dbg["P1"], P1[:])
            nc.sync.dma_start(dbg["C3in"], C3in[:])
            nc.sync.dma_start(dbg["H30"], H3[0][:])
            nc.sync.dma_start(dbg["havg0"], havg[0][:])
            nc.sync.dma_start(dbg["havg1"], havg[1][:])


_NC_CACHE = []


def kernel(**inputs):
    if not _NC_CACHE:
        _NC_CACHE.append(_build_nc())
    nc = _NC_CACHE[0]
    w = _prep_weights(inputs)
    x = np.asarray(inputs["x"], np.float32)
    in_maps = []
    for c in range(NCORES):
        m = dict(w)
        m["x"] = np.ascontiguousarray(x[c * BSH : (c + 1) * BSH])
        in_maps.append(m)
    res = run_bass_kernel_spmd(nc, in_maps, list(range(NCORES))).results
    out = np.concatenate([res[c]["y"].reshape(BSH, 1) for c in range(NCORES)], 0)
    return out


# revision 44
# speedup vs baseline: 1.3042x; 1.0711x over previous
"""Trainium2 Bass kernel for nn_CNN_MAMBA2 (CNN + Mamba2(L=1) + MLP head).

Strategy: pure data parallel over batch (B=256 -> 32 per core x 8 cores).
Each core runs the full network on its batch shard; weights are replicated.

Layouts (per core, bh = 32 batches x 2 rows = 64 independent 1D signals):
  X    [64, 3936]   batch-major padded input (xpad[i] = x[i-25])
  Xp   [128, 7680]  position-major: built with 120 PE transposes.
  conv1: fp32r matmuls as before; maxpool(4) as DVE 4:1 tensor_reduce from
         PSUM into bf16 staging M1; BN+ReLU+fp8 cast on the Pool engine
         (2 tensor_scalar ops) writing P1.
  P1   [128, 8320]  pooled, fp8e4 (x16 scale), partition = 64*delta + ci
  conv2: fp8e4 DoubleRow matmuls. Tap pairs (jp, jp+1) are 64-col shifts of
         P1, expressed as one overlapping AP with a 2-wide k-tile dim.
         Weights x32 in fp8, optionally split hi+lo for precision.
  C3in [128, 8192]  conv2 out, fp8e4 (x16 scale), written by Act evacuation
  conv3: fp8e4 DoubleRow, same trick (taps are 64-col shifts).
  H3   2 x [128, 3840] bf16 conv3 out; avgpool = DVE reduce (2x mode on
         bf16) -> feature-major h [256, 32] bf16
  Mamba2 with L=1: single scan step from h0=0 =>
         y = xin * (dt * (B.C) + D) (per head), gated RMSNorm, out_proj, MLP.
  Feature-major mamba; partition reductions/broadcasts via ones-matmuls.

Host-side prep is layout-only plus dtype casts and exact power-of-2
scalings of weights (fp8 range placement); BN folding and the matching
1/2^k compensation happen on device via constant multiplier columns.
"""

import numpy as np

import bass_rust
import concourse.bass as bass
import concourse.mybir as mybir
from concourse import masks
from concourse.ap import AP
from concourse.tile import TileContext
from concourse.bass_utils import run_bass_kernel_spmd

F32 = mybir.dt.float32
F32R = mybir.dt.float32r
BF16 = mybir.dt.bfloat16
F8 = mybir.dt.float8e4
AF = mybir.ActivationFunctionType
ALU = mybir.AluOpType
AX = mybir.AxisListType
PM = mybir.MatmulPerfMode

EPS = 1e-5
NCORES = 8
BSH = 32            # batches per core
BH = 64             # bh signals per core
NC1 = 120           # C blocks (conv1 output pairs / pool blocks)
XPAD = 3936

DEBUG_DUMPS = False

# conv1 chunks whose PSUM evacuation goes through the Activation engine
# instead of a DVE tensor_reduce (load balancing)
ACT_PATH = {1, 3, 5, 7, 9, 11, 13}

# hi+lo fp8 weight split per conv (False = single fp8 chain, 2x faster on
# PE but ~3.6% weight-quantization error; True = ~0.1%)
W2_SPLIT = True
W3_SPLIT = True
NCH2 = 2 if W2_SPLIT else 1
NCH3 = 2 if W3_SPLIT else 1
S1 = 16.0           # P1 fp8 scale
S3A = 16.0          # C3in fp8 scale
S2W = 32.0          # conv2 weight fp8 scale
S3W = 32.0          # conv3 weight fp8 scale

NP8 = mybir.dt.np(F8)


def _split_multi_waits(nc):
    """This walrus build accepts at most one sync-wait command per
    instruction; Tile's sem assignment attaches several. Hoist extra waits
    onto dedicated single-wait nops right before the instruction (same
    engine), which preserves blocking semantics."""
    n = 0
    for fn in nc.m.functions:
        for bb in fn.blocks:
            out = []
            for inst in bb.instructions:
                si = inst.sync_info
                waits = list(si.on_wait) if si is not None else []
                if len(waits) > 1:
                    for w in waits[:-1]:
                        n += 1
                        nop = mybir.InstNoOp(name=f"waitnop-{n}", ins=[], outs=[])
                        nop.engine = inst.engine
                        nop.debug = inst.debug
                        nop.sync_info = bass_rust.SyncInfo(
                            on_wait=[w], on_update=[]
                        )
                        out.append(nop)
                    si.on_wait = [waits[-1]]
                    inst.sync_info = si
                out.append(inst)
            bb.instructions = out


def _sview(t, col0, dims):
    """Overlapping strided free-dim view of a [128, W] tile: dims is a list
    of [stride, size] free dims, partition dim kept."""
    base = t[:, col0 : col0 + 1]
    return AP(base.tensor, base.offset, [list(base.ap[0])] + dims)


# --------------------------------------------------------------------------
# host-side weight prep (layout / cast / exact power-of-2 scaling only)
# --------------------------------------------------------------------------

def _quant_pairs(full, scale, n_pairs, split):
    """full: [128, ntap, 128] f32. Returns [128, nch * n_pairs * 2 * 128]
    fp8 with tap pairs (2q, 2q+1) stacked as DoubleRow k-tiles, missing
    taps zero, lo chain = residual after hi quantization."""
    k, ntap, m = full.shape
    sc = (full * scale).astype(np.float32)
    hi = sc.astype(NP8)
    chains = [hi]
    if split:
        lo = (sc - hi.astype(np.float32)).astype(NP8)
        chains.append(lo)
    out = np.zeros((k, len(chains), n_pairs, 2, m), NP8)
    for c, arr in enumerate(chains):
        for q in range(n_pairs):
            for i in range(2):
                t = 2 * q + i
                if t < ntap:
                    out[:, c, q, i, :] = arr[:, t, :]
    return out.reshape(k, -1)


def _prep_weights(inp):
    f32 = np.float32
    c1w = np.asarray(inp["c1w"], f32).reshape(64, 51)
    # lhsT for conv1: K rows are input positions c relative to the 32-position
    # chunk base; column m = 128*j is absorbed by leading 4j zero rows so the
    # rhs can always start at partition 0 (PE base-partition constraint).
    w1t = np.zeros((79, 4, 128), f32)
    for j in range(4):
        for d in range(2):
            for c in range(4 * j + 16 * d, 4 * j + 16 * d + 51):
                w1t[c, j, 64 * d : 64 * d + 64] = c1w[:, c - 4 * j - 16 * d]
    w1t = w1t.reshape(79, 512)

    c2w = np.asarray(inp["c2w"], f32).reshape(128, 64, 21)
    w2full = np.zeros((128, 11, 128), f32)
    for jp in range(11):
        for d in range(2):
            t = 2 * jp + d
            if t <= 20:
                w2full[64 * d : 64 * d + 64, jp, :] = c2w[:, :, t].T
    w2t = _quant_pairs(w2full, S2W, 6, W2_SPLIT)

    c3w = np.asarray(inp["c3w"], f32).reshape(256, 128, 9)
    w3full = np.zeros((128, 2 * 9, 128), f32)
    for hf in range(2):
        for k in range(9):
            w3full[:, 9 * hf + k, :] = c3w[128 * hf : 128 * hf + 128, :, k].T
    # pack per hf separately (pairs within each hf's 9 taps)
    w3t = np.concatenate(
        [
            _quant_pairs(w3full[:, 9 * hf : 9 * hf + 9, :], S3W, 5, W3_SPLIT)
            for hf in range(2)
        ],
        axis=1,
    )

    mw_in = np.asarray(inp["mw_in"], f32)          # [1160, 256]
    w_inT = np.zeros((128, 2, 1160), f32)
    for k in range(2):
        w_inT[:, k, :] = mw_in[:, 128 * k : 128 * k + 128].T
    w_inT = w_inT.astype(np.dtype(mybir.dt.np(BF16)))

    mw_out = np.asarray(inp["mw_out"], f32)        # [256, 512]
    w_outT = np.zeros((128, 4, 2, 128), f32)
    for k in range(4):
        for m in range(2):
            w_outT[:, k, m, :] = mw_out[
                128 * m : 128 * m + 128, 128 * k : 128 * k + 128
            ].T

    f1w = np.asarray(inp["f1w"], f32)              # [64, 256]
    f1wT = np.zeros((128, 2, 64), f32)
    for k in range(2):
        f1wT[:, k, :] = f1w[:, 128 * k : 128 * k + 128].T

    f2wT = np.asarray(inp["f2w"], f32).reshape(1, 64).T.copy()   # [64, 1]

    def t2(a):
        return np.tile(np.asarray(a, f32), 2)

    def pd(a):
        a = np.asarray(a, f32)
        return np.pad(a, (0, 128 - a.shape[0]))

    vecs = np.zeros((128, 54), f32)
    # cols 0-4 bn gammas, 5-9 betas, 10-14 means, 15-19 vars, 20-24 pre-bias
    vecs[:, 0] = t2(inp["bn1g"]); vecs[:, 5] = t2(inp["bn1b"])
    vecs[:, 10] = t2(inp["bn1m"]); vecs[:, 15] = t2(inp["bn1v"])
    vecs[:, 20] = t2(inp["c1b"])
    vecs[:, 1] = inp["bn2g"]; vecs[:, 6] = inp["bn2b"]
    vecs[:, 11] = inp["bn2m"]; vecs[:, 16] = inp["bn2v"]
    vecs[:, 21] = inp["c2b"]
    for hf in range(2):
        s = slice(128 * hf, 128 * hf + 128)
        vecs[:, 2 + hf] = inp["bn3g"][s]; vecs[:, 7 + hf] = inp["bn3b"][s]
        vecs[:, 12 + hf] = inp["bn3m"][s]; vecs[:, 17 + hf] = inp["bn3v"][s]
        vecs[:, 22 + hf] = inp["c3b"][s]
    vecs[:, 4] = pd(inp["bn4g"]); vecs[:, 9] = pd(inp["bn4b"])
    vecs[:, 14] = pd(inp["bn4m"]); vecs[:, 19] = pd(inp["bn4v"])
    vecs[:, 24] = pd(inp["f1b"])
    vecs[0:8, 25] = inp["mdt_bias"]
    vecs[0:8, 26] = inp["mD"]
    vecs[0:1, 27] = inp["f2b"]
    mcw = np.asarray(inp["mconv_w"], f32)[:, 0, 3]
    mcb = np.asarray(inp["mconv_b"], f32)
    vecs[:, 28:33] = mcw.reshape(5, 128).T
    vecs[:, 33:38] = mcb.reshape(5, 128).T
    vecs[:, 38:42] = np.asarray(inp["mnorm_w"], f32).reshape(4, 128).T
    vecs[0:64, 42] = mcw[576:640]
    vecs[0:64, 43] = mcb[576:640]
    # constant fp8-scale compensation multipliers for BN scale/shift
    vecs[:, 44:49] = np.array(
        [S1, S3A / (S1 * S2W), 1.0 / (S3A * S3W), 1.0 / (S3A * S3W), 1.0],
        f32,
    )[None, :]
    vecs[:, 49:54] = np.array([S1, S3A, 1.0, 1.0, 1.0], f32)[None, :]

    # constant head-expansion matrix: emat[h, 128*t + m] = 1 iff h == 2t + m//64
    emat = np.zeros((8, 512), f32)
    for t in range(4):
        emat[2 * t, 128 * t : 128 * t + 64] = 1.0
        emat[2 * t + 1, 128 * t + 64 : 128 * t + 128] = 1.0

    return {
        "w1t": w1t, "w2t": w2t, "w3t": w3t,
        "w_inT": w_inT.reshape(128, -1), "w_outT": w_outT.reshape(128, -1),
        "f1wT": f1wT.reshape(128, -1), "f2wT": f2wT, "vecs": vecs, "emat": emat,
    }


# --------------------------------------------------------------------------
# device kernel
# --------------------------------------------------------------------------

def _build_nc():
    nc = bass.Bass("TRN2", target_bir_lowering=False, debug=False)

    x_d = nc.dram_tensor("x", [BSH, 2, 3840], F32, kind="ExternalInput").ap()
    w1t_d = nc.dram_tensor("w1t", [79, 512], F32R, kind="ExternalInput").ap()
    w2t_d = nc.dram_tensor("w2t", [128, NCH2 * 6 * 256], F8,
                           kind="ExternalInput").ap()
    w3t_d = nc.dram_tensor("w3t", [128, NCH3 * 2 * 5 * 256], F8,
                           kind="ExternalInput").ap()
    w_inT_d = nc.dram_tensor("w_inT", [128, 2 * 1160], BF16,
                             kind="ExternalInput").ap()
    w_outT_d = nc.dram_tensor("w_outT", [128, 1024], F32, kind="ExternalInput").ap()
    f1wT_d = nc.dram_tensor("f1wT", [128, 128], F32, kind="ExternalInput").ap()
    f2wT_d = nc.dram_tensor("f2wT", [64, 1], F32, kind="ExternalInput").ap()
    vecs_d = nc.dram_tensor("vecs", [128, 54], F32, kind="ExternalInput").ap()
    emat_d = nc.dram_tensor("emat", [8, 512], F32, kind="ExternalInput").ap()
    y_d = nc.dram_tensor("y", [1, BSH], F32, kind="ExternalOutput").ap()
    dbg = {}
    if DEBUG_DUMPS:
        dbg["P1"] = nc.dram_tensor("dbg_P1", [128, 130 * 64], F8,
                                   kind="ExternalOutput").ap()
        dbg["C3in"] = nc.dram_tensor("dbg_C3in", [128, 128 * 64], F8,
                                     kind="ExternalOutput").ap()
        dbg["H30"] = nc.dram_tensor("dbg_H30", [128, 60 * 64], BF16,
                                    kind="ExternalOutput").ap()
        dbg["havg0"] = nc.dram_tensor("dbg_havg0", [128, BSH], BF16,
                                      kind="ExternalOutput").ap()
        dbg["havg1"] = nc.dram_tensor("dbg_havg1", [128, BSH], BF16,
                                      kind="ExternalOutput").ap()
        dbg["dts"] = nc.dram_tensor("dbg_dts", [8, BSH], F32,
                                    kind="ExternalOutput").ap()
        dbg["g"] = nc.dram_tensor("dbg_g", [8, BSH], F32,
                                  kind="ExternalOutput").ap()
        dbg["y0"] = nc.dram_tensor("dbg_y0", [128, BSH], F32,
                                   kind="ExternalOutput").ap()
        dbg["osb"] = nc.dram_tensor("dbg_osb", [128, 64], F32,
                                    kind="ExternalOutput").ap()
        dbg["xcB"] = nc.dram_tensor("dbg_xcB", [64, BSH], F32,
                                    kind="ExternalOutput").ap()

    with TileContext(nc) as tc:
        _body(nc, tc, x_d, w1t_d, w2t_d, w3t_d, w_inT_d, w_outT_d,
              f1wT_d, f2wT_d, vecs_d, emat_d, y_d, dbg)
    _split_multi_waits(nc)
    return nc


def _body(nc, tc, x_d, w1t_d, w2t_d, w3t_d, w_inT_d, w_outT_d,
          f1wT_d, f2wT_d, vecs_d, emat_d, y_d, dbg=None):
    with (
        tc.tile_pool(name="pw", bufs=1) as pw,
        tc.tile_pool(name="pmain", bufs=1) as pm,
        tc.tile_pool(name="ptmp", bufs=3) as pt,
        tc.tile_pool(name="pp", bufs=1, space="PSUM") as pp,
    ):
        # ---- X: padded batch-major input, loaded in chunks so transposes
        # can start before the whole shard lands ----
        X = pm.tile([64, XPAD], F32)
        ident = pw.tile([64, 64], F32)
        masks.make_identity(nc, ident[:])
        nc.vector.memset(X[:, 0:25], 0.0)
        xflat = x_d.rearrange("b h w -> (b h) w")
        xcuts = [0, 160, 352, 640, 1600, 2720, 3840]
        for c in range(6):
            w0, w1 = xcuts[c], xcuts[c + 1]
            nc.sync.dma_start(X[:, 25 + w0 : 25 + w1], xflat[:, w0:w1])
        nc.vector.memset(X[:, 3865:XPAD], 0.0)

        w1t = pw.tile([79, 512], F32R)
        nc.scalar.dma_start(w1t[:], w1t_d)
        vecs = pw.tile([128, 54], F32)
        nc.scalar.dma_start(vecs[:], vecs_d)

        # ---- T / T2: position-major via PE transposes (stride 64) ----
        T = pm.tile([128, 60 * 64], F32R)
        T2 = pm.tile([128, 60 * 64], F32R)
        P1 = pm.tile([128, 130 * 64], F8)
        nc.vector.memset(P1[:, 0:320], 0.0)
        nc.vector.memset(P1[:, 8000:8320], 0.0)
        C3in = pm.tile([128, 128 * 64], F8)
        nc.vector.memset(C3in[:, 0:256], 0.0)
        nc.vector.memset(C3in[:, 7936:8192], 0.0)
        H3 = [pm.tile([128, 60 * 64], BF16, tag=f"h3_{i}", name=f"h3_{i}")
              for i in range(2)]
        havg = [pm.tile([128, BSH], BF16, tag=f"havg_{i}", name=f"havg_{i}")
                for i in range(2)]

        def tgroup(Tt, off, g):
            nd = 8 if g < 7 else 4
            tp = pp.tile([128, 512], F32, tag="mm", bufs=2, name="tp")
            for d in range(nd):
                D = 8 * g + d
                nc.tensor.transpose(
                    tp[:, 64 * d : 64 * d + 64],
                    X[:, 64 * D + off : 64 * D + off + 128], ident[:],
                )
            nc.scalar.copy(
                Tt[:, 512 * g : 512 * g + 64 * nd], tp[:, : 64 * nd]
            )

        ones_col = pw.tile([128, 1], F32)
        nc.vector.memset(ones_col[:], 1.0)
        ones_row = pw.tile([1, 128], F32)
        nc.vector.memset(ones_row[:], 1.0)
        ones_s8 = pw.tile([64, 8], F32)
        nc.vector.memset(ones_s8[:], 1.0)
        eps_col = pw.tile([1, 1], F32)
        nc.vector.memset(eps_col[:], EPS)

        # remaining weights (issued after X so they don't delay transposes)
        w2t = pw.tile([128, NCH2 * 6 * 256], F8)
        nc.gpsimd.dma_start(w2t[:], w2t_d)
        w3t = pw.tile([128, NCH3 * 2 * 5 * 256], F8)
        nc.gpsimd.dma_start(w3t[:], w3t_d)
        w_inT = pw.tile([128, 2 * 1160], BF16)
        nc.sync.dma_start(w_inT[:], w_inT_d)
        w_outT = pw.tile([128, 1024], F32)
        nc.sync.dma_start(w_outT[:], w_outT_d)
        f1wT = pw.tile([128, 128], F32)
        nc.sync.dma_start(f1wT[:], f1wT_d)
        f2wT = pw.tile([64, 1], F32)
        nc.sync.dma_start(f2wT[:], f2wT_d)
        emat = pw.tile([8, 512], F32)
        nc.sync.dma_start(emat[:], emat_d)
        # ---- BN scale/bias precompute: s = g/sqrt(v+eps); c = (b0-m)*s+beta
        # then fp8-range compensation via constant multiplier columns
        s_all = pw.tile([128, 5], F32)
        c_all = pw.tile([128, 5], F32)
        tmpv = pw.tile([128, 5], F32)
        nc.vector.tensor_scalar_add(tmpv[:], vecs[:, 15:20], EPS)
        nc.scalar.sqrt(tmpv[:], tmpv[:])
        nc.vector.reciprocal(tmpv[:], tmpv[:])
        nc.vector.tensor_mul(s_all[:], vecs[:, 0:5], tmpv[:])
        nc.vector.tensor_sub(tmpv[:], vecs[:, 20:25], vecs[:, 10:15])
        nc.vector.tensor_mul(tmpv[:], tmpv[:], s_all[:])
        nc.vector.tensor_add(c_all[:], tmpv[:], vecs[:, 5:10])
        nc.vector.tensor_mul(s_all[:], s_all[:], vecs[:, 44:49])
        nc.vector.tensor_mul(c_all[:], c_all[:], vecs[:, 49:54])

        # ---- conv1 + maxpool(4) + bn + relu ----
        # out w = 8C + j + 4*delta; C = 2D (+1 odd); rhs cols (D, bh)
        # DVE-path: DVE 4:1 reduce PSUM -> M1 bf16; BN+ReLU+fp8 on Pool.
        # Act-path: Act evacuates j-halves with BN+ReLU fused (commutes
        # with max), DVE does two bf16 2x-mode tensor_tensor maxes.
        p1v = P1[:].rearrange("p (c b) -> p c b", b=64)

        def conv1_chunk(n):
            act_path = n in ACT_PATH
            M1 = None
            if not act_path:
                M1 = pt.tile([128, 512], BF16, tag="m1", name="m1")
                m1v = M1[:].rearrange("p (c b) -> p c b", b=64)
            for par in range(2):
                Tt = T if par == 0 else T2
                idx = (2 * n + par) % 3
                if idx < 2:
                    ps = pp.tile([128, 1024], F32, tag="c1", bufs=2, name="c1")
                else:
                    ps = pp.tile([128, 1024], F32, tag="acc", bufs=1, name="c1a")
                cs = slice(256 * n, 256 * n + 256)
                for j in range(4):
                    nc.tensor.matmul(
                        ps[:, 256 * j : 256 * j + 256],
                        w1t[:, 128 * j : 128 * j + 128],
                        Tt[0:79, cs], start=True, stop=True,
                    )
                if act_path:
                    a0 = pt.tile([128, 512], BF16, tag="a0", name="a0")
                    a1 = pt.tile([128, 512], BF16, tag="a1", name="a1")
                    nc.scalar.activation(a0[:], ps[:, 0:512], AF.Relu,
                                         bias=c_all[:, 0:1], scale=s_all[:, 0:1])
                    nc.scalar.activation(a1[:], ps[:, 512:1024], AF.Relu,
                                         bias=c_all[:, 0:1], scale=s_all[:, 0:1])
                    u = pt.tile([128, 512], BF16, tag="u", name="u")
                    nc.vector.tensor_tensor(u[:], a0[:], a1[:], ALU.max)
                    nc.vector.tensor_tensor(
                        p1v[:, 8 * n + 5 + par : 8 * n + 13 + par : 2, :],
                        u[:, 0:256], u[:, 256:512], ALU.max,
                    )
                else:
                    nc.vector.tensor_reduce(
                        m1v[:, par:8:2, :],
                        ps[:].rearrange("p (j x) -> p x j", j=4),
                        AX.X, ALU.max,
                    )
            if not act_path:
                nc.gpsimd.tensor_scalar(
                    M1[:], M1[:], s_all[:, 0:1], c_all[:, 0:1], ALU.mult, ALU.add
                )
                nc.gpsimd.tensor_scalar(
                    P1[:, (8 * n + 5) * 64 : (8 * n + 5) * 64 + 512],
                    M1[:], 0.0, None, ALU.max,
                )

        def conv2_chunk(n):
            ps = pp.tile([128, 512], F32, tag="mm", bufs=2, name="c2")
            nmm = NCH2 * 6
            mi = 0
            for ch in range(NCH2):
                for q in range(6):
                    stride = 64 if q < 5 else 0
                    rhs = _sview(P1, (8 * n + 2 * q) * 64,
                                 [[stride, 2], [1, 512]])
                    lhsT = w2t[:, (ch * 6 + q) * 256 : (ch * 6 + q) * 256 + 256]
                    nc.tensor.matmul(
                        ps[:], lhsT.rearrange("p (i m) -> p i m", i=2), rhs,
                        start=(mi == 0), stop=(mi == nmm - 1),
                        perf_mode=PM.DoubleRow,
                    )
                    mi += 1
            nc.scalar.activation(
                C3in[:, 256 + 512 * n : 256 + 512 * n + 512], ps[:],
                AF.Relu, bias=c_all[:, 1:2], scale=s_all[:, 1:2],
            )

        chunks3 = [(8 * i, 8) for i in range(7)] + [(56, 4)]

        def conv3_chunk(hf, ci):
            v0, nv = chunks3[ci]
            ps = pp.tile([128, 512], F32, tag="mm", bufs=2, name="c3")
            out_ap = ps[:, : nv * 64]
            nmm = NCH3 * 5
            mi = 0
            for ch in range(NCH3):
                for q in range(5):
                    rhs = _sview(C3in, (2 * v0 + 2 * q) * 64,
                                 [[64, 2], [128, nv], [1, 64]])
                    off = ((hf * NCH3 + ch) * 5 + q) * 256
                    lhsT = w3t[:, off : off + 256]
                    nc.tensor.matmul(
                        out_ap, lhsT.rearrange("p (i m) -> p i m", i=2), rhs,
                        start=(mi == 0), stop=(mi == nmm - 1),
                        perf_mode=PM.DoubleRow,
                    )
                    mi += 1
            nc.scalar.activation(
                H3[hf][:, 64 * v0 : 64 * (v0 + nv)], out_ap,
                AF.Relu, bias=c_all[:, 2 + hf : 3 + hf],
                scale=s_all[:, 2 + hf : 3 + hf],
            )

        def conv3_avg(hf, ci):
            v0, nv = chunks3[ci]
            hv = H3[hf][:, 64 * v0 : 64 * (v0 + nv)].rearrange(
                "p (v b h) -> p b v h", v=nv, b=32, h=2
            )
            with nc.allow_low_precision(reason="avgpool partials in bf16"):
                if ci == 0:
                    nc.vector.tensor_reduce(havg[hf][:], hv, AX.XY, ALU.add)
                else:
                    hp = pt.tile([128, BSH], BF16, tag="hp", name="hp")
                    nc.vector.tensor_reduce(hp[:], hv, AX.XY, ALU.add)
                    nc.vector.tensor_add(havg[hf][:], havg[hf][:], hp[:])


        def weight_prescale():
            # fold avgpool 1/120 into in_proj weights; fold gated-RMSNorm
            # weight into out_proj contraction rows (one-time DVE work,
            # emitted at a mid-pipeline DVE lull)
            with nc.allow_low_precision(reason="bf16 weight prescale"):
                nc.vector.tensor_scalar_mul(w_inT[:], w_inT[:], 1.0 / 120.0)
            for k in range(4):
                nc.vector.tensor_scalar_mul(
                    w_outT[:, 256 * k : 256 * k + 256],
                    w_outT[:, 256 * k : 256 * k + 256],
                    vecs[:, 38 + k : 39 + k],
                )

        # interleaved emission: conv1(n) -> conv2(n-3) -> conv3(hf0, ...)
        state = {"e1": 0, "e2": 0, "e3": 0, "ea": 0, "wps": False}

        def pump():
            if state["e1"] >= 7 and not state["wps"]:
                state["wps"] = True
                weight_prescale()
            while state["e2"] <= state["e1"] - 3 and state["e2"] < 15:
                conv2_chunk(state["e2"])
                state["e2"] += 1
                while state["e3"] < 8 and 2 * state["e3"] + 3 <= state["e2"] - 1:
                    conv3_chunk(0, state["e3"])
                    conv3_chunk(1, state["e3"])
                    state["e3"] += 1
                    while state["ea"] <= state["e3"] - 3:
                        conv3_avg(0, state["ea"])
                        conv3_avg(1, state["ea"])
                        state["ea"] += 1

        for g in range(8):
            tgroup(T, 0, g)
            tgroup(T2, 32, g)
            while state["e1"] <= 2 * g - 1 and state["e1"] < 15:
                conv1_chunk(state["e1"])
                state["e1"] += 1
                pump()
        while state["e1"] < 15:
            conv1_chunk(state["e1"])
            state["e1"] += 1
            pump()
        while state["e2"] < 15:
            conv2_chunk(state["e2"])
            state["e2"] += 1
            while state["e3"] < 8 and 2 * state["e3"] + 3 <= state["e2"] - 1:
                conv3_chunk(0, state["e3"])
                conv3_chunk(1, state["e3"])
                state["e3"] += 1
        while state["e3"] < 8:
            conv3_chunk(0, state["e3"])
            conv3_chunk(1, state["e3"])
            state["e3"] += 1

        for ci in range(state["ea"], 8):
            conv3_avg(0, ci)
            conv3_avg(1, ci)

        # in_proj: M-tiles (z:0-3, xBC, dt), K=2x128, bf16 weights/acts
        ip = pp.tile([128, 352], F32, tag="c1", bufs=2, name="ip")
        mtiles = [(10, 1152, 8), (8, 1024, 64), (9, 1088, 64)]
        mtiles += [(m, 128 * m, 128) for m in range(4, 8)]
        mtiles += [(m, 128 * m, 128) for m in range(4)]
        for m, f0, mm in mtiles:
            for k in range(2):
                nc.tensor.matmul(
                    ip[0:mm, 32 * m : 32 * m + 32],
                    w_inT[:, 1160 * k + f0 : 1160 * k + f0 + mm],
                    havg[k][:],
                    start=(k == 0), stop=(k == 1),
                )

        # ---- mamba + classifier (feature-major, batch on free dim) ----
        xcB = pt.tile([64, BSH], F32, tag="xcB")
        nc.scalar.activation(
            xcB[:], ip[0:64, 256:288], AF.Silu,
            bias=vecs[0:64, 37:38], scale=vecs[0:64, 32:33],
        )
        xcC = pt.tile([64, BSH], F32, tag="xcC")
        nc.scalar.activation(
            xcC[:], ip[0:64, 288:320], AF.Silu,
            bias=vecs[0:64, 43:44], scale=vecs[0:64, 42:43],
        )
        dts = pt.tile([8, BSH], F32, tag="dts")
        # softplus(x + b) = ln(1 + exp(x + b)) (no Softplus ACT table here)
        nc.scalar.activation(
            dts[:], ip[0:8, 320:352], AF.Exp, bias=vecs[0:8, 25:26]
        )
        nc.scalar.activation(dts[:], dts[:], AF.Ln, bias=1.0)
        xc = [pt.tile([128, BSH], F32, tag=f"xc{m}", name=f"xc{m}") for m in range(4)]
        for m in range(4):
            nc.scalar.activation(
                xc[m][:], ip[:, 32 * (4 + m) : 32 * (4 + m) + 32], AF.Silu,
                bias=vecs[:, 33 + m : 34 + m], scale=vecs[:, 28 + m : 29 + m],
            )
        zsall = pt.tile([128, 4 * BSH], F32, tag="zsall")
        nc.scalar.activation(zsall[:], ip[:, 0:128], AF.Silu)
        zs = [zsall[:, 32 * m : 32 * m + 32] for m in range(4)]

        # s = sum_f Bm*Cm (per batch scalar) replicated on 8 partitions,
        # via a single ones-matmul with an all-ones [64, 8] lhsT
        bc = pt.tile([64, BSH], F32, tag="bc")
        nc.vector.tensor_mul(bc[:], xcB[:], xcC[:])
        ps_s8 = pp.tile([8, BSH], F32, tag="mm", bufs=2, name="ps_s8")
        nc.tensor.matmul(ps_s8[:], ones_s8[:], bc[:], start=True, stop=True)
        g = pt.tile([8, BSH], F32, tag="g")
        nc.vector.tensor_mul(g[:], dts[:], ps_s8[:])
        nc.vector.tensor_scalar_add(g[:], g[:], vecs[0:8, 26:27])

        y = [pt.tile([128, BSH], F32, tag=f"y{t}", name=f"y{t}") for t in range(4)]
        ps_ms = pp.tile([1, BSH], F32, tag="c1", bufs=2, name="ps_ms")
        for t in range(4):
            ge = pp.tile([128, BSH], F32, tag="mm", bufs=2, name="ge")
            nc.tensor.matmul(ge[:], emat[:, 128 * t : 128 * t + 128], g[:],
                             start=True, stop=True)
            nc.vector.tensor_mul(y[t][:], xc[t][:], ge[:])
            nc.vector.tensor_mul(y[t][:], y[t][:], zs[t])
            sq = pt.tile([128, BSH], F32, tag="sq")
            nc.vector.tensor_mul(sq[:], y[t][:], y[t][:])
            nc.tensor.matmul(ps_ms[:], ones_col[:], sq[:],
                             start=(t == 0), stop=(t == 3))
        # out_proj [256,512] @ y -> o (2 M-tiles in one psum); the RMSNorm
        # 1/rms(y) is per-batch so it commutes through the matmul and is
        # applied once on the [128, 64] result. norm_w is folded into
        # w_outT, so out_proj runs concurrently with the sqrt/recip chain.
        ps_o = pp.tile([128, 64], F32, tag="mm", bufs=2, name="ps_o")
        for m in range(2):
            for k in range(4):
                nc.tensor.matmul(
                    ps_o[:, 32 * m : 32 * m + 32],
                    w_outT[:, (k * 2 + m) * 128 : (k * 2 + m) * 128 + 128],
                    y[k][:],
                    start=(k == 0), stop=(k == 3),
                )
        sd = pt.tile([1, BSH], F32, tag="sd")
        nc.scalar.activation(sd[:], ps_ms[:], AF.Sqrt,
                             bias=eps_col[:], scale=1.0 / 512.0)
        rinv = pt.tile([1, BSH], F32, tag="rinv")
        nc.vector.reciprocal(rinv[:], sd[:])
        ps_rb = pp.tile([128, BSH], F32, tag="c1", bufs=2, name="ps_rb")
        nc.tensor.matmul(ps_rb[:], ones_row[:], rinv[:], start=True, stop=True)
        rb_sb = pt.tile([128, BSH], F32, tag="rb_sb")
        nc.vector.tensor_copy(rb_sb[:], ps_rb[:])
        o_sb = pt.tile([128, 64], F32, tag="o_sb")
        for m in range(2):
            nc.vector.tensor_mul(
                o_sb[:, 32 * m : 32 * m + 32],
                ps_o[:, 32 * m : 32 * m + 32], rb_sb[:],
            )

        # fc1 + bn4 + relu
        ps_f1 = pp.tile([64, BSH], F32, tag="c1", bufs=2, name="ps_f1")
        for k in range(2):
            nc.tensor.matmul(
                ps_f1[:], f1wT[:, 64 * k : 64 * k + 64],
                o_sb[:, 32 * k : 32 * k + 32],
                start=(k == 0), stop=(k == 1),
            )
        o1 = pt.tile([64, BSH], F32, tag="o1")
        nc.scalar.activation(o1[:], ps_f1[:], AF.Relu,
                             bias=c_all[0:64, 4:5], scale=s_all[0:64, 4:5])

        # fc2
        ps_f2 = pp.tile([1, BSH], F32, tag="c1", bufs=2, name="ps_f2")
        nc.tensor.matmul(ps_f2[:], f2wT[:], o1[:], start=True, stop=True)
        ores = pt.tile([1, BSH], F32, tag="ores")
        nc.scalar.activation(ores[:], ps_f2[:], AF.Identity,
                             bias=vecs[0:1, 27:28])
        nc.sync.dma_start(y_d, ores[:])
        if dbg:
            nc.sync.dma_start(dbg["P1"], P1[:])
            nc.sync.dma_start(dbg["C3in"], C3in[:])
            nc.sync.dma_start(dbg["H30"], H3[0][:])
            nc.sync.dma_start(dbg["havg0"], havg[0][:])
            nc.sync.dma_start(dbg["havg1"], havg[1][:])
            nc.sync.dma_start(dbg["dts"], dts[:])
            nc.sync.dma_start(dbg["g"], g[:])
            nc.sync.dma_start(dbg["y0"], y[0][:])
            nc.sync.dma_start(dbg["osb"], o_sb[:])
            nc.sync.dma_start(dbg["xcB"], xcB[:])


_NC_CACHE = []


def kernel(**inputs):
    if not _NC_CACHE:
        _NC_CACHE.append(_build_nc())
    nc = _NC_CACHE[0]
    w = _prep_weights(inputs)
    x = np.asarray(inputs["x"], np.float32)
    in_maps = []
    for c in range(NCORES):
        m = dict(w)
        m["x"] = np.ascontiguousarray(x[c * BSH : (c + 1) * BSH])
        in_maps.append(m)
    res = run_bass_kernel_spmd(nc, in_maps, list(range(NCORES))).results
    out = np.concatenate([res[c]["y"].reshape(BSH, 1) for c in range(NCORES)], 0)
    return out
